# revision 19
# baseline (speedup 1.0000x reference)
"""TRN2 Bass kernel for nn_DUST_65085934403760 (topk_masking).

Computes, per the reference:
  B  = x @ W_d^T                                  (2048, 2048)
  z1 = top50(B); z2 = top50(B + z1 @ S^T)
  u  = (z2 @ W_d) @ W_d
  att[p,b] = <prev[p,b,:], u[b,:]>  -> softmax over batch axis (global)
  z3 = lambda2 * sum_p clip(prev[p]) * att[p]
  z  = top50(B + z @ S^T)  x10
  mD = sum_b (z[:, :1024]^2 + z[:, 1024:]^2); normalized to [0,1]
Returns (mD_norm, z_last).

Sharding: batch axis across 8 NeuronCores (256 rows each); W_d/S replicated.
S^T stays resident in SBUF (16 MB); the global-batch softmax uses one tiny
AllGather of per-core (max, sum) stats with the standard rescale trick.
All matmuls run in true fp32 (PE hi/lo passes): the top-k support selection
flips on ~1e-5 perturbations, so bf16/fp32r matmuls would corrupt the output.

Pipeline notes: each LIHT round emits per m-tile [16 PE transposes -> n-outer
K-accumulation matmuls -> v=B+y adds], so m0's top-k search (DVE) hides under
m1's matmuls (PE); n-outer frees each PSUM bank right after its K-loop so the
next m-tile never waits on a whole-phase barrier. z lives in-place in v.
"""
import numpy as np

BATCH = 2048
D = 2048
P = 8
OMEGA = 50
N_ITERS = 10
CLAMP = 150.0
N_CORES = 8
BSH = BATCH // N_CORES          # 256 batch rows per core
MT = BSH // 128                 # 2 m-tiles
KT = D // 128                   # 16 k-tiles
NT = D // 512                   # 4 n-tiles (fp32 moving-operand max)
W_HALF = D // 2                 # 1024

_CACHE = {}
RUN_KWARGS = {}      # test harness can set {"trace": True} for NTFF profiling
LAST_RESULT = None   # BassKernelResults of the most recent kernel() call


def _build(debug=False):
    import concourse.bacc as bacc
    import concourse.mybir as mybir
    import concourse.tile as tile
    from concourse import masks

    f32 = mybir.dt.float32
    AF = mybir.ActivationFunctionType
    OP = mybir.AluOpType

    nc = bacc.Bacc("TRN2", target_bir_lowering=False, debug=False,
                   num_devices=N_CORES)

    xT_d = nc.dram_tensor("xT", [D, BSH], f32, kind="ExternalInput")
    WdT_d = nc.dram_tensor("WdT", [D, D], f32, kind="ExternalInput")
    Wd_d = nc.dram_tensor("Wd", [D, D], f32, kind="ExternalInput")
    St_d = nc.dram_tensor("St", [D, D], f32, kind="ExternalInput")
    prev_d = nc.dram_tensor("prev", [P, BSH, D], f32, kind="ExternalInput")
    pwc_d = nc.dram_tensor("pwc", [P, BSH, D], f32, kind="ExternalInput")
    z_out = nc.dram_tensor("z_out", [BSH, D], f32, kind="ExternalOutput")
    md_out = nc.dram_tensor("mD_part", [1, W_HALF], f32, kind="ExternalOutput")
    dbg = {}
    if debug:
        for name, shape in [("dbg_B", [BSH, D]), ("dbg_z2", [BSH, D]),
                            ("dbg_z3", [BSH, D]), ("dbg_att", [P, BSH]),
                            ("dbg_w", [P, BSH])]:
            dbg[name] = nc.dram_tensor(name, shape, f32, kind="ExternalOutput")

    with tile.TileContext(nc) as tc:
        with (
            tc.tile_pool(name="pers", bufs=1) as pers,
            tc.tile_pool(name="scratch", bufs=3) as scr,
            tc.tile_pool(name="small", bufs=4) as small,
            tc.tile_pool(name="dram", bufs=1, space="DRAM") as dramp,
        ):
            ident = pers.tile([128, 128], f32)
            masks.make_identity(nc, ident[:])

            # S^T fully resident: St_sb[p, k, :] = S^T[k*128 + p, :]
            St_sb = pers.tile([128, KT, D], f32)
            for k in range(KT):
                nc.sync.dma_start(St_sb[:, k, :], St_d[k * 128:(k + 1) * 128, :])

            B_sb = [pers.tile([128, D], f32, tag=f"B{m}", name=f"B{m}")
                    for m in range(MT)]
            # v doubles as z (top-k masks in place); also holds s_t / scratch
            v_sb = [pers.tile([128, D], f32, tag=f"v{m}", name=f"v{m}")
                    for m in range(MT)]
            lhsT = [pers.tile([128, KT, 128], f32, tag=f"lhsT{m}",
                              name=f"lhsT{m}") for m in range(MT)]

            att4 = pers.tile([128, MT, P, NT], f32)   # per-n dot partials
            att_sb = pers.tile([128, MT, P], f32)
            attP = pers.tile([P, BSH], f32)
            eP = attP  # exp applied in place
            wT_sb = pers.tile([128, MT, P], f32)
            sall = pers.tile([P, N_CORES, 2], f32)

            cc_in = dramp.tile([P, 2], f32)
            cc_out = dramp.tile([N_CORES, P, 2], f32)

            def topk_mask(m, vsrc, zdst):
                """zdst = vsrc masked to its OMEGA largest-|.| entries.

                zdst may alias vsrc (the final select streams in place).
                """
                aP = scr.tile([128, D], f32, tag="slab", name="aP")
                a2 = scr.tile([128, D], f32, tag="slab", name="a2")
                nc.scalar.activation(aP[:], vsrc[:], AF.Abs)
                nc.scalar.activation(a2[:], vsrc[:], AF.Abs)
                n_rounds = (OMEGA + 7) // 8
                t8 = None
                for r in range(n_rounds):
                    t8 = small.tile([128, 8], f32, tag="t8", name="t8")
                    nc.vector.max(t8[:], a2[:])
                    if r < n_rounds - 1:
                        nc.vector.match_replace(a2[:], t8[:], a2[:], -1.0)
                kidx = (OMEGA - 1) - 8 * (n_rounds - 1)
                nc.vector.scalar_tensor_tensor(
                    zdst[:], aP[:], t8[:, kidx:kidx + 1], vsrc[:],
                    op0=OP.is_ge, op1=OP.mult)

            def consume_v(m, n, ps):
                nc.vector.tensor_tensor(
                    v_sb[m][:, n * 512:(n + 1) * 512], ps[:],
                    B_sb[m][:, n * 512:(n + 1) * 512], op=OP.add)

            def liht_round(name, consume=consume_v):
                """One z -> B + z@S^T pass, m-interleaved, z read from v_sb."""
                with (
                    tc.tile_pool(name=name + "t", bufs=4, space="PSUM") as trp,
                    tc.tile_pool(name=name + "y", bufs=1, space="PSUM") as myp,
                ):
                    for m in range(MT):
                        for k in range(KT):
                            tp = trp.tile([128, 128], f32, tag="tp", name="tp")
                            nc.tensor.transpose(
                                tp[:], v_sb[m][:, k * 128:(k + 1) * 128],
                                ident[:])
                            nc.scalar.copy(lhsT[m][:, k, :], tp[:])
                        for n in range(NT):
                            ps = myp.tile([128, 512], f32, tag=f"y{n}",
                                          name=f"y{n}")
                            for k in range(KT):
                                nc.tensor.matmul(
                                    ps[:], lhsT[m][:, k, :],
                                    St_sb[:, k, n * 512:(n + 1) * 512],
                                    start=(k == 0), stop=(k == KT - 1))
                            consume(m, n, ps)

            def stream_phase(rhs_rows_fn, consume, name):
                """K-accumulation with DRAM-streamed rhs slabs (k-outer)."""
                with tc.tile_pool(name=name, bufs=1, space="PSUM") as mmps:
                    ps = [[mmps.tile([128, 512], f32, tag=f"y{m}{n}",
                                     name=f"y{m}{n}")
                           for n in range(NT)] for m in range(MT)]
                    for k in range(KT):
                        wsl = scr.tile([128, D], f32, tag="slab", name="wsl")
                        nc.sync.dma_start(wsl[:], rhs_rows_fn(k))
                        for m in range(MT):
                            for n in range(NT):
                                nc.tensor.matmul(
                                    ps[m][n][:], lhsT[m][:, k, :],
                                    wsl[:, n * 512:(n + 1) * 512],
                                    start=(k == 0), stop=(k == KT - 1))
                    for m in range(MT):
                        for n in range(NT):
                            consume(m, n, ps[m][n])

            def transpose_v_to_lhsT(name):
                with tc.tile_pool(name=name, bufs=4, space="PSUM") as trp:
                    for m in range(MT):
                        for k in range(KT):
                            tp = trp.tile([128, 128], f32, tag="tp", name="tp")
                            nc.tensor.transpose(
                                tp[:], v_sb[m][:, k * 128:(k + 1) * 128],
                                ident[:])
                            nc.scalar.copy(lhsT[m][:, k, :], tp[:])

            # ---------------- B = x @ W_d^T ----------------
            for m in range(MT):
                for k in range(KT):
                    nc.sync.dma_start(
                        lhsT[m][:, k, :],
                        xT_d[k * 128:(k + 1) * 128, m * 128:(m + 1) * 128])

            def consume_B(m, n, ps):
                nc.scalar.copy(B_sb[m][:, n * 512:(n + 1) * 512], ps[:])

            stream_phase(lambda k: WdT_d[k * 128:(k + 1) * 128, :],
                         consume_B, "psB")
            if debug:
                for m in range(MT):
                    nc.sync.dma_start(
                        dbg["dbg_B"][m * 128:(m + 1) * 128, :], B_sb[m][:])

            # ---------------- z1 = top50(B) (into v) ----------------
            for m in range(MT):
                topk_mask(m, B_sb[m], v_sb[m])

            # ---------------- z2 = top50(B + z1 @ S^T) ----------------
            liht_round("psz2")
            for m in range(MT):
                topk_mask(m, v_sb[m], v_sb[m])
            if debug:
                for m in range(MT):
                    nc.sync.dma_start(
                        dbg["dbg_z2"][m * 128:(m + 1) * 128, :], v_sb[m][:])

            # ------------- s_t = z2 @ W_d (into v, via lhsT) -------------
            transpose_v_to_lhsT("trz2")

            def consume_st(m, n, ps):
                nc.scalar.copy(v_sb[m][:, n * 512:(n + 1) * 512], ps[:])

            stream_phase(lambda k: Wd_d[k * 128:(k + 1) * 128, :],
                         consume_st, "psst")

            # ------- u = s_t @ W_d (stays in PSUM) + att dot products -------
            transpose_v_to_lhsT("trst")
            with tc.tile_pool(name="psu", bufs=1, space="PSUM") as ups:
                ups_t = [[ups.tile([128, 512], f32, tag=f"u{m}{n}",
                                   name=f"u{m}{n}")
                          for n in range(NT)] for m in range(MT)]
                for k in range(KT):
                    wsl = scr.tile([128, D], f32, tag="slab", name="wsl")
                    nc.sync.dma_start(wsl[:], Wd_d[k * 128:(k + 1) * 128, :])
                    for m in range(MT):
                        for n in range(NT):
                            nc.tensor.matmul(
                                ups_t[m][n][:], lhsT[m][:, k, :],
                                wsl[:, n * 512:(n + 1) * 512],
                                start=(k == 0), stop=(k == KT - 1))
                # att partial dots straight off PSUM (junk elementwise out
                # goes to v_sb; s_t there is dead after the transposes)
                for p in range(P):
                    for m in range(MT):
                        psl = scr.tile([128, D], f32, tag="slab", name="psl")
                        nc.sync.dma_start(
                            psl[:], prev_d[p, m * 128:(m + 1) * 128, :])
                        for n in range(NT):
                            nc.vector.scalar_tensor_tensor(
                                v_sb[m][:, n * 512:(n + 1) * 512],
                                ups_t[m][n][:], 1.0,
                                psl[:, n * 512:(n + 1) * 512],
                                op0=OP.mult, op1=OP.mult,
                                accum_out=att4[:, m, p, n:n + 1])
            for m in range(MT):
                nc.vector.tensor_reduce(
                    att_sb[:, m, :], att4[:, m, :, :],
                    axis=mybir.AxisListType.X, op=OP.add)

            # transpose att -> [p, b] layout
            with tc.tile_pool(name="tratt", bufs=2, space="PSUM") as trps:
                for m in range(MT):
                    tp = trps.tile([P, 128], f32, tag="tpa", name="tpa")
                    nc.tensor.transpose(tp[:], att_sb[:, m, :], ident[:])
                    nc.scalar.copy(attP[:, m * 128:(m + 1) * 128], tp[:])
            if debug:
                nc.sync.dma_start(dbg["dbg_att"][:], attP[:])

            # local softmax stats + global rescale via AllGather
            mloc = small.tile([P, 1], f32, tag="mloc", name="mloc")
            nc.vector.tensor_reduce(mloc[:], attP[:], axis=mybir.AxisListType.X,
                                    op=OP.max)
            negm = small.tile([P, 1], f32, tag="negm", name="negm")
            nc.vector.tensor_scalar_mul(negm[:], mloc[:], -1.0)
            sloc = small.tile([P, 1], f32, tag="sloc", name="sloc")
            nc.scalar.activation(eP[:], attP[:], AF.Exp, bias=negm[:],
                                 scale=1.0, accum_out=sloc[:])
            nc.sync.dma_start(cc_in[:, 0:1], mloc[:])
            nc.sync.dma_start(cc_in[:, 1:2], sloc[:])
            nc.gpsimd.collective_compute(
                "AllGather", OP.bypass,
                replica_groups=[list(range(N_CORES))],
                ins=[cc_in.opt()], outs=[cc_out.opt()])
            nc.sync.dma_start(sall[:], cc_out[:].rearrange("c p t -> p c t"))

            M8 = small.tile([P, 1], f32, tag="M8", name="M8")
            nc.vector.tensor_reduce(M8[:], sall[:, :, 0],
                                    axis=mybir.AxisListType.X, op=OP.max)
            negM8 = small.tile([P, 1], f32, tag="negM8", name="negM8")
            nc.vector.tensor_scalar_mul(negM8[:], M8[:], -1.0)
            ex8 = small.tile([P, N_CORES], f32, tag="ex8", name="ex8")
            nc.scalar.activation(ex8[:], sall[:, :, 0], AF.Exp, bias=negM8[:])
            scr8 = small.tile([P, N_CORES], f32, tag="scr8", name="scr8")
            Ssum = small.tile([P, 1], f32, tag="Ssum", name="Ssum")
            nc.vector.scalar_tensor_tensor(
                scr8[:], ex8[:], 1.0, sall[:, :, 1],
                op0=OP.mult, op1=OP.mult, accum_out=Ssum[:])
            # scale = exp(mloc - M8) / Ssum
            dm = small.tile([P, 1], f32, tag="dm", name="dm")
            nc.vector.tensor_sub(dm[:], mloc[:], M8[:])
            edm = small.tile([P, 1], f32, tag="edm", name="edm")
            nc.scalar.activation(edm[:], dm[:], AF.Exp)
            rS = small.tile([P, 1], f32, tag="rS", name="rS")
            nc.vector.reciprocal(rS[:], Ssum[:])
            scl = small.tile([P, 1], f32, tag="scl", name="scl")
            nc.vector.tensor_mul(scl[:], edm[:], rS[:])
            nc.vector.tensor_scalar_mul(eP[:], eP[:], scl[:, 0:1])
            if debug:
                nc.sync.dma_start(dbg["dbg_w"][:], eP[:])

            # transpose weights back to [b, p] layout
            with tc.tile_pool(name="trw", bufs=2, space="PSUM") as trps:
                for m in range(MT):
                    tp = trps.tile([128, P], f32, tag="tpw", name="tpw")
                    nc.tensor.transpose(
                        tp[:], eP[:, m * 128:(m + 1) * 128], ident[0:P, 0:P])
                    nc.scalar.copy(wT_sb[:, m, :], tp[:])

            # ---------- z3 = sum_p pwc[p] * w[p] (into v) ----------
            for m in range(MT):
                for p in range(P):
                    psl = scr.tile([128, D], f32, tag="slab", name="psl")
                    nc.sync.dma_start(
                        psl[:], pwc_d[p, m * 128:(m + 1) * 128, :])
                    if p == 0:
                        nc.vector.tensor_scalar_mul(
                            v_sb[m][:], psl[:], wT_sb[:, m, p:p + 1])
                    else:
                        nc.vector.scalar_tensor_tensor(
                            v_sb[m][:], psl[:], wT_sb[:, m, p:p + 1],
                            v_sb[m][:], op0=OP.mult, op1=OP.add)
            if debug:
                for m in range(MT):
                    nc.sync.dma_start(
                        dbg["dbg_z3"][m * 128:(m + 1) * 128, :], v_sb[m][:])

            # -------- LIHT iterations (z in place in v) --------
            for it in range(N_ITERS):
                liht_round(f"ps{it}")
                for m in range(MT):
                    topk_mask(m, v_sb[m], v_sb[m])

            # -------- epilogue: z out, mD partial --------
            for m in range(MT):
                nc.sync.dma_start(z_out[m * 128:(m + 1) * 128, :], v_sb[m][:])

            # squares in place (the z_out DMA above reads v first)
            for m in range(MT):
                nc.scalar.square(v_sb[m][:, 0:W_HALF], v_sb[m][:, 0:W_HALF])
                nc.scalar.square(v_sb[m][:, W_HALF:D], v_sb[m][:, W_HALF:D])
                nc.vector.tensor_tensor(v_sb[m][:, 0:W_HALF],
                                        v_sb[m][:, 0:W_HALF],
                                        v_sb[m][:, W_HALF:D], op=OP.add)
            # ones vector for the batch-sum matmul lives in the (dead) lhsT
            ones = lhsT[0][:, 0, 0:1]
            nc.gpsimd.memset(ones, 1.0)
            with tc.tile_pool(name="psmd", bufs=1, space="PSUM") as mdps:
                mdp = [mdps.tile([1, 512], f32, tag=f"md{h}", name=f"md{h}")
                       for h in range(2)]
                for m in range(MT):
                    for h in range(2):
                        nc.tensor.matmul(
                            mdp[h][:], ones,
                            v_sb[m][:, h * 512:(h + 1) * 512],
                            start=(m == 0), stop=(m == MT - 1))
                # stage through a dead corner of lhsT[1] (flattened view)
                md_sb = lhsT[1][0:1, 0:8, :].rearrange("p a b -> p (a b)")
                for h in range(2):
                    nc.scalar.copy(md_sb[:, h * 512:(h + 1) * 512], mdp[h][:])
                nc.sync.dma_start(md_out[:], md_sb[:])

    nc.compile()
    return nc


def _get_program(debug=False):
    key = ("prog", debug)
    if key not in _CACHE:
        _CACHE[key] = _build(debug=debug)
    return _CACHE[key]


def kernel(x, prev_windows, W_d, S, lambda2):
    from concourse.bass_utils import run_bass_kernel_spmd

    x = np.asarray(x, np.float32)
    prev_windows = np.asarray(prev_windows, np.float32)
    W_d = np.asarray(W_d, np.float32)
    S = np.asarray(S, np.float32)
    lam = np.float32(np.asarray(lambda2))

    nc = _get_program()
    WdT = np.ascontiguousarray(W_d.T)
    St = np.ascontiguousarray(S.T)
    in_maps = []
    for c in range(N_CORES):
        sl = slice(c * BSH, (c + 1) * BSH)
        prev_c = np.ascontiguousarray(prev_windows[:, sl, :])
        pwc_c = (np.clip(prev_c, -CLAMP, CLAMP) * lam).astype(np.float32)
        in_maps.append({
            "xT": np.ascontiguousarray(x[sl].T),
            "WdT": WdT, "Wd": W_d, "St": St,
            "prev": prev_c, "pwc": pwc_c,
        })
    res = run_bass_kernel_spmd(nc, in_maps, list(range(N_CORES)), **RUN_KWARGS)
    global LAST_RESULT
    LAST_RESULT = res
    z_full = np.concatenate([res.results[c]["z_out"] for c in range(N_CORES)],
                            axis=0)
    mD = np.sum([res.results[c]["mD_part"][0] for c in range(N_CORES)],
                axis=0, dtype=np.float32)
    mD_norm = ((mD - mD.min()) / (mD.max() - mD.min() + np.float32(1e-8)))
    return mD_norm.astype(np.float32), z_full
